# revision 2
# baseline (speedup 1.0000x reference)
"""Trainium2 Bass kernel: out = x @ ((W_int + offset) * scale), fp8 DoubleRow.

Math: V = W - 63 (zero-mean ints, |V| <= 63), cast to fp8 e4m3 (E[dV^2]~0.88);
x cast to fp8 e4m3 (rms rel err ~2.6%). Then
  out[m,n] = scale[n] * ((x8 @ V8)[m,n] + (63 + offset[n]) * rowsum(x)[m])
The rank-1 term uses the exact f32 rowsum, so W's mean and the offset are
exact; total rel err ~1.84e-2 (measured vs f64 on the actual inputs).

PE: DoubleRow fp8 matmuls (2 fp8 per cell along K, 2x MACs/cycle).
Orientation: W is stationary ([128k, 2slot, 128n] per (kp, nb)), x^T is
moving ([128k, 2slot, 512m] chunks) -> output transposed [n, m] in PSUM.
Each (nb, m-quarter) group accumulates 16 k-pairs x 2 half-chunks = 32 MMs
into a [128, 1024] PSUM tile; epilogue: DVE adds the rank-1 term
(s_bcast * offc), ACT applies the per-n scale to SBUF, DMA to a transposed
DRAM output [1408, 4096] (host un-transposes and crops).

Sharding: column-parallel over N across 8 cores (NSH=1376 each).
Warmup: first 4 nb-groups of quarter 0 run kp-interleaved so the PE tracks
W/x DMA arrival; x quarters are double-buffered, W (5.5MB fp8) persistent.
"""

import numpy as np
import ml_dtypes

M, K, N = 4096, 4096, 11008
NCORES = 8
NSH = N // NCORES          # 1376
P = 128
KP = 16                    # k-pairs of 256
NB = 11                    # n-blocks: 10x128 + 96
NBW = [128] * 10 + [96]
NQ = 4                     # m-quarters
MQW = 1024                 # m-quarter width
NBPAD = NB * P             # 1408 padded rows of transposed out

_E4 = ml_dtypes.float8_e4m3

_cache = {}


def _build_nc():
    import concourse.bacc as bacc
    import concourse.mybir as mybir
    import concourse.tile as tile

    fp8 = mybir.dt.float8e4
    f32 = mybir.dt.float32
    DR = mybir.MatmulPerfMode.DoubleRow
    Copy = mybir.ActivationFunctionType.Copy

    nc = bacc.Bacc(None, target_bir_lowering=False)
    # xq rows: (q*KP + kp)*P + p ; cols: slot*MQW + m  (x^T in fp8 pairs)
    xq = nc.dram_tensor("xq", [NQ * KP * P, 2 * MQW], fp8, kind="ExternalInput")
    # wq rows: kp*P + p ; cols: slot*NSH + n  (V8 = e4m3(W-63) pairs)
    wq = nc.dram_tensor("wq", [KP * P, 2 * NSH], fp8, kind="ExternalInput")
    sbc = nc.dram_tensor("sbc", [P, M], f32, kind="ExternalInput")      # rowsum bcast
    offc = nc.dram_tensor("offc", [P, NB], f32, kind="ExternalInput")   # 63+offset
    scalec = nc.dram_tensor("scalec", [P, NB], f32, kind="ExternalInput")
    outt = nc.dram_tensor("outt", [NBPAD, M], f32, kind="ExternalOutput")

    xq3 = xq.ap().rearrange("(g p) f -> p g f", p=P)        # [128, NQ*KP, 2048]
    wq3 = wq.ap().rearrange("(kp p) f -> p kp f", p=P)      # [128, KP, 2752]
    outt3 = outt.ap().rearrange("(nb p) m -> p nb m", p=P)  # [128, NB, 4096]

    with tile.TileContext(nc) as tc:
        with (
            tc.tile_pool(name="wpool", bufs=KP) as wpool,
            tc.tile_pool(name="xpool", bufs=2 * KP) as xpool,
            tc.tile_pool(name="cpool", bufs=1) as cpool,
            tc.tile_pool(name="opool", bufs=4) as opool,
            tc.tile_pool(name="psp", bufs=4, space="PSUM") as psp,
        ):
            # W: all 16 kp tiles, persistent. even->gpsimd, odd->scalar.
            w_sb = []
            for kp in range(KP):
                t = wpool.tile([P, 2, NSH], fp8, tag="w", name=f"w{kp}")
                eng = nc.gpsimd if kp % 2 == 0 else nc.scalar
                eng.dma_start(
                    t[:], wq3[:, kp, :].rearrange("p (s n) -> p s n", s=2)
                )
                w_sb.append(t)

            x_tiles = {}

            def load_xq(q):
                for kp in range(KP):
                    t = xpool.tile([P, 2, MQW], fp8, tag="x", name=f"x{q}_{kp}")
                    nc.sync.dma_start(
                        t[:],
                        xq3[:, q * KP + kp, :].rearrange(
                            "p (s m) -> p s m", s=2
                        ),
                    )
                    x_tiles[(q, kp)] = t

            load_xq(0)

            # constants: rowsum-bcast split per quarter (q0 needed first)
            sbc_sb = cpool.tile([P, M], f32, tag="sbc")
            nc.sync.dma_start(sbc_sb[:, 0:MQW], sbc.ap()[:, 0:MQW])
            offc_sb = cpool.tile([P, NB], f32, tag="offc")
            nc.gpsimd.dma_start(offc_sb[:], offc.ap())
            scalec_sb = cpool.tile([P, NB], f32, tag="scalec")
            nc.gpsimd.dma_start(scalec_sb[:], scalec.ap())

            def mm(ps, q, nb, kp):
                nbw = NBW[nb]
                for h in range(2):
                    nc.tensor.matmul(
                        ps[:nbw, h * 512:(h + 1) * 512],
                        w_sb[kp][:, :, nb * P:nb * P + nbw],
                        x_tiles[(q, kp)][:, :, h * 512:(h + 1) * 512],
                        start=(kp == 0),
                        stop=(kp == KP - 1),
                        perf_mode=DR,
                    )

            def epilogue(ps, q, nb):
                nbw = NBW[nb]
                # ps += (63 + offset[n]) * rowsum_x[m]
                nc.vector.scalar_tensor_tensor(
                    ps[:nbw, :],
                    sbc_sb[:nbw, q * MQW:(q + 1) * MQW],
                    offc_sb[:nbw, nb:nb + 1],
                    ps[:nbw, :],
                    mybir.AluOpType.mult,
                    mybir.AluOpType.add,
                )
                o_sb = opool.tile([P, MQW], f32, tag="o")
                # o = ps * scale[n]
                nc.scalar.activation(
                    o_sb[:nbw, :],
                    ps[:nbw, :],
                    Copy,
                    scale=scalec_sb[:nbw, nb:nb + 1],
                )
                nc.gpsimd.dma_start(
                    outt3[:nbw, nb, q * MQW:(q + 1) * MQW], o_sb[:nbw, :]
                )

            # Phase A: quarter 0, nb 0..3 kp-interleaved (tracks DMA arrival).
            ps_a = [
                psp.tile([P, MQW], f32, tag="ps", name=f"psA{g}")
                for g in range(4)
            ]
            for kp in range(KP):
                for g in range(4):
                    mm(ps_a[g], 0, g, kp)
            for g in range(4):
                epilogue(ps_a[g], 0, g)

            # Phase B/C: remaining groups, x quarters prefetched one ahead.
            for q in range(NQ):
                nb0 = 4 if q == 0 else 0
                for nb in range(nb0, NB):
                    if q + 1 < NQ and nb == nb0:
                        load_xq(q + 1)
                        if q == 0:
                            nc.sync.dma_start(
                                sbc_sb[:, MQW:], sbc.ap()[:, MQW:]
                            )
                    ps = psp.tile([P, MQW], f32, tag="ps")
                    for kp in range(KP):
                        mm(ps, q, nb, kp)
                    epilogue(ps, q, nb)
    nc.compile()
    return nc


def _get_nc():
    if "nc" not in _cache:
        _cache["nc"] = _build_nc()
    return _cache["nc"]


def _prep_inputs(x, weight, antiquant_scale, antiquant_offset):
    x = np.asarray(x, dtype=np.float32)
    weight = np.asarray(weight)
    scale = np.asarray(antiquant_scale, dtype=np.float32)
    off = np.asarray(antiquant_offset, dtype=np.float32)

    x8t = x.astype(_E4).T                      # [K, M] fp8
    xdr = np.ascontiguousarray(
        x8t.reshape(KP, 2, P, NQ, MQW).transpose(3, 0, 2, 1, 4)
    ).reshape(NQ * KP * P, 2 * MQW)
    rs = x.astype(np.float64).sum(axis=1).astype(np.float32)
    sbc = np.ascontiguousarray(np.broadcast_to(rs[None, :], (P, M)))

    V8 = (weight.astype(np.float32) - 63.0).astype(_E4)   # [K, N]

    in_maps = []
    for c in range(NCORES):
        sl = slice(c * NSH, (c + 1) * NSH)
        wdr = np.ascontiguousarray(
            V8[:, sl].reshape(KP, 2, P, NSH).transpose(0, 2, 1, 3)
        ).reshape(KP * P, 2 * NSH)
        opad = np.zeros(NBPAD, dtype=np.float32)
        opad[:NSH] = 63.0 + off[sl]
        spad = np.zeros(NBPAD, dtype=np.float32)
        spad[:NSH] = scale[sl]
        in_maps.append({
            "xq": xdr,
            "wq": wdr,
            "sbc": sbc,
            "offc": np.ascontiguousarray(opad.reshape(NB, P).T),
            "scalec": np.ascontiguousarray(spad.reshape(NB, P).T),
        })
    return in_maps


def kernel(x, weight, antiquant_scale, antiquant_offset, _trace=False):
    from concourse.bass_utils import run_bass_kernel_spmd

    nc = _get_nc()
    in_maps = _prep_inputs(x, weight, antiquant_scale, antiquant_offset)
    res = run_bass_kernel_spmd(
        nc, in_maps, core_ids=list(range(NCORES)), trace=_trace
    )
    out = np.concatenate(
        [np.asarray(res.results[c]["outt"])[:NSH] for c in range(NCORES)],
        axis=0,
    )
    if _trace:
        _cache["last_result"] = res
    return np.ascontiguousarray(out.T).astype(np.float32)
